# revision 25
# baseline (speedup 1.0000x reference)
"""Factored (column) attention kernel for Trainium2, 8 NeuronCores.

Reference computation (B=4, S=4096, D=1024, BLOCK_LEN=128, NB=32):
    qkv = x @ Wqkv + bqkv ; split q,k,v
    'column' attention: each (batch, within-block position bl) row attends
    causally over the NB=32 block indices -> 512 independent length-32
    single-head attentions with head dim 1024.
    out = attn @ Wout + bout

Algebraic fold (halves device matmul work vs the 4-GEMM formulation):
  scores = (x Wq + bq) . (x Wk)  =  (x M + bq_eff) . x,  M = Wq Wk^T,
           bq_eff = Wk bq           (bk cancels in softmax)
  out    = p @ (x Wv + bv) Wout + bout = p @ (x N) + bo_eff,
           N = Wv Wout, bo_eff = bout + bv Wout   (rows of p sum to 1)
so the device only computes TWO [tok,1024]x[1024,1024] GEMMs (q'' = x M,
vout = x N) plus the tiny 32-long attentions; k/v/out projections vanish.
M, N, bq_eff, bo_eff are precomputed host-side in fp32.

Sharding: data-parallel over the 512 independent (b, bl) attention rows,
64 rows (2048 tokens) per core.  All operands are staged host-side into
[128, wide] layouts so each DRAM transfer is one large fully-contiguous
DMA (dma_start issue costs ~0.7us of engine time each, and the startup
is HBM-bandwidth-bound, so few+fat+critical-first transfers matter):
  - x^T block-major: one [128, 4096] (=1MB) DMA per 512-token block
  - M, N: [128, 8192] tiles, DMA'd in quarters/halves interleaved with
    x^T block 0 across the three DMA-capable queues (sync/act/gpsimd),
    critical first; subtile deps let the PE start on the first quarter
  - q'' produced in transposed layout [D, tok] (lhsT = M chunk)
  - scores for a 4-group q-pack: one [K=128,M=128,N=128] matmul per
    d-chunk of q''^T against the SAME x^T chunk (no k projection)
  - softmax on [128,128] tiles; exp+rowsum fused via accum_out;
    normalized p transposed by one DVE stream-transpose
  - vout = x N in natural layout [tok, D] (lhsT = x^T chunk)
  - out = p @ vout via lhsT = p^T: psum [128 q-tok, 512 d] natural, so
    each q-pack's output is one 256KB fully-contiguous DMA; bo_eff is
    added host-side
Numerics: all matmul operands fp16 (fp32 PSUM accumulation); host-
simulated end-to-end rms error vs the fp32 reference is ~5.3e-4.
"""

import numpy as np

import concourse.bacc as bacc
import concourse.mybir as mybir
import concourse.tile as tile
from concourse.bass_utils import run_bass_kernel_spmd

N_CORES = 8
B, S, D = 4, 4096, 1024
BL = 128          # BLOCK_LEN (within-block positions)
NB = S // BL      # 32 block indices = attention sequence length
NGROUP = B * BL   # 512 independent attention rows
GPC = NGROUP // N_CORES   # 64 groups per core
TOK = GPC * NB    # 2048 tokens per core
BLK = 512         # tokens per fused block (16 groups, 4 q-packs)
NBLK = TOK // BLK  # 4
QP = BLK // 128   # q-packs per block
DC = D // 128     # 8 d-chunks
XW = DC * BLK     # 4096 cols of one x^T block tile
SCALE = 1.0 / np.sqrt(D)
NEG = -1.0e30
WARMUP = 26       # PE warm-up matmuls covering the first weight DMAs

F32 = mybir.dt.float32
F16 = mybir.dt.float16

_PROGRAM = None


def _get_program():
    global _PROGRAM
    if _PROGRAM is None:
        _PROGRAM = _build_program()
    return _PROGRAM


def _build_program():
    nc = bacc.Bacc("TRN2", target_bir_lowering=False, debug=False,
                   num_devices=N_CORES)
    xt = nc.dram_tensor("xt", [NBLK * 128, XW], F16,
                        kind="ExternalInput").ap()
    wm = nc.dram_tensor("wm", [128, DC * D], F16, kind="ExternalInput").ap()
    wn = nc.dram_tensor("wn", [128, DC * D], F16, kind="ExternalInput").ap()
    bq = nc.dram_tensor("bq", [D], F32, kind="ExternalInput").ap()
    mask = nc.dram_tensor("mask", [128, 128], F32,
                          kind="ExternalInput").ap()
    ot = nc.dram_tensor("ot", [TOK, D], F16, kind="ExternalOutput").ap()

    with tile.TileContext(nc) as tc:
        with (
            tc.tile_pool(name="w", bufs=1) as w_pool,
            tc.tile_pool(name="const", bufs=1) as const,
            tc.tile_pool(name="xt", bufs=4) as xt_pool,
            tc.tile_pool(name="q", bufs=2) as q_pool,
            tc.tile_pool(name="v", bufs=5) as v_pool,
            tc.tile_pool(name="sm", bufs=6) as sm_pool,
            tc.tile_pool(name="smh", bufs=8) as smh_pool,
            tc.tile_pool(name="small", bufs=8) as small_pool,
            tc.tile_pool(name="out", bufs=3) as out_pool,
            tc.tile_pool(name="psA", bufs=5, space="PSUM") as psA,
            tc.tile_pool(name="psB", bufs=3, space="PSUM") as psB,
        ):
            # --- staged input DMAs.  warm-up matmuls on a zeroed tile
            # keep the PE busy (and its clock ramped up) while the first
            # weight DMAs land
            wu = const.tile([128, 512], F16, tag="warm")
            nc.vector.memset(wu[:], 0.0)
            wu_ps = psB.tile([128, 512], F32, tag="psB", name="wu_ps")
            for _ in range(WARMUP):
                nc.tensor.matmul(wu_ps[:], lhsT=wu[:, 0:128], rhs=wu[:],
                                 start=True, stop=True)
            # critical bytes (M + x^T block 0 = 3MB) first, ~1MB per DMA
            # queue.  M is staged host-side in j-strip order so each
            # 256KB strip completes one q'' output chunk's weights; the
            # PE's j-psum pipeline starts after strip 0 + x^T block 0
            # instead of after the whole 2MB of M.
            wm_sb = w_pool.tile([128, DC * D], F16, tag="wm", name="wm_sb")
            wn_sb = w_pool.tile([128, DC * D], F16, tag="wn", name="wn_sb")
            xt_sbs = [xt_pool.tile([128, XW], F16, tag="xt", name=f"xt{b}")
                      for b in range(NBLK)]
            mask_sb = const.tile([128, 128], F32, tag="mask")
            bq_sb = const.tile([128, DC], F32, tag="bq")
            # gpsimd's SWDGE ring is ~4x slower than the sync/scalar
            # HWDGE rings -- it only gets the tiny constants; everything
            # else rides the two fast rings, critical bytes first
            nc.gpsimd.dma_start(mask_sb[:], mask[:])
            nc.gpsimd.dma_start(bq_sb[:], bq.rearrange("(c p) -> p c", p=128))
            nc.sync.dma_start(xt_sbs[0][:, 0:XW // 2],
                              xt[0:128, 0:XW // 2])
            nc.scalar.dma_start(xt_sbs[0][:, XW // 2:XW],
                                xt[0:128, XW // 2:XW])
            # strip 0 split across both rings: the first q'' psum only
            # waits for x^T block 0 + 128KB per ring beyond it
            nc.sync.dma_start(wm_sb[:, 0:D // 2], wm[:, 0:D // 2])
            nc.scalar.dma_start(wm_sb[:, D // 2:D], wm[:, D // 2:D])
            for j in range(1, DC):
                eng = nc.sync if j % 2 == 0 else nc.scalar
                eng.dma_start(wm_sb[:, D * j:D * (j + 1)],
                              wm[:, D * j:D * (j + 1)])
            # non-critical: N halves, x^T block 1 (blocks 2,3 prefetch
            # from inside the block loop)
            H = DC * D // 2
            nc.sync.dma_start(wn_sb[:, 0:H], wn[:, 0:H])
            nc.scalar.dma_start(wn_sb[:, H:2 * H], wn[:, H:2 * H])
            nc.sync.dma_start(xt_sbs[1][:, 0:XW // 2], xt[128:256, 0:XW // 2])
            nc.scalar.dma_start(xt_sbs[1][:, XW // 2:XW],
                                xt[128:256, XW // 2:XW])

            def wmS(c, j):
                return wm_sb[:, 1024 * j + 128 * c:1024 * j + 128 * (c + 1)]

            def wnS(c, h):
                return wn_sb[:, 1024 * c + 512 * h:1024 * c + 512 * (h + 1)]

            for b in range(NBLK):
                if b + 2 < NBLK:
                    r0 = 128 * (b + 2)
                    nc.sync.dma_start(xt_sbs[b + 2][:, 0:XW // 2],
                                      xt[r0:r0 + 128, 0:XW // 2])
                    nc.scalar.dma_start(xt_sbs[b + 2][:, XW // 2:XW],
                                        xt[r0:r0 + 128, XW // 2:XW])
                xt_sb = xt_sbs[b]

                def xtS(c, lo, hi):
                    return xt_sb[:, 512 * c + lo:512 * c + hi]

                # --- q''^T projection: psum [dout-chunk 128, BLK tok]
                q_sb = []
                for j in range(DC):
                    ps = psA.tile([128, BLK], F32, tag="psA")
                    for c in range(DC):
                        nc.tensor.matmul(
                            ps[:], lhsT=wmS(c, j), rhs=xtS(c, 0, BLK),
                            start=(c == 0), stop=(c == DC - 1),
                        )
                    q = q_pool.tile([128, BLK], F16, tag=f"q{j}",
                                    name=f"q{j}")
                    nc.scalar.add(q[:], ps[:], bq_sb[:, j:j + 1])
                    q_sb.append(q)

                # --- scores + softmax per 4-group q-pack (before the
                # vout projection so the softmax chain hides behind it)
                pt_sb = []
                for qp in range(QP):
                    ps = psB.tile([128, 128], F32, tag="psB")
                    for c in range(DC):
                        nc.tensor.matmul(
                            ps[:],
                            lhsT=q_sb[c][:, 128 * qp:128 * (qp + 1)],
                            rhs=xtS(c, 128 * qp, 128 * (qp + 1)),
                            start=(c == 0), stop=(c == DC - 1),
                        )
                    tm = sm_pool.tile([128, 128], F32, tag="sm")
                    nc.vector.tensor_add(tm[:], ps[:], mask_sb[:])
                    p4 = sm_pool.tile([128, 128], F32, tag="sm")
                    s4 = small_pool.tile([128, 1], F32, tag="s4")
                    nc.scalar.activation(
                        p4[:], tm[:], mybir.ActivationFunctionType.Exp,
                        scale=float(SCALE), accum_out=s4[:],
                    )
                    r4 = small_pool.tile([128, 1], F32, tag="r4")
                    nc.vector.reciprocal(r4[:], s4[:])
                    pn = smh_pool.tile([128, 128], F16, tag="smh")
                    nc.vector.tensor_scalar_mul(pn[:], p4[:], r4[:])
                    pt = smh_pool.tile([128, 128], F16, tag="smh")
                    nc.vector.transpose(pt[:], pn[:])
                    pt_sb.append(pt)

                # --- vout = x N natural: psum [tok-chunk 128, 512 dout]
                def _vout(tch):
                    vt = v_pool.tile([128, D], F16, tag="v", name="vt")
                    for hh in range(2):
                        ps = psA.tile([128, 512], F32, tag="psA")
                        for c in range(DC):
                            nc.tensor.matmul(
                                ps[:],
                                lhsT=xtS(c, 128 * tch, 128 * (tch + 1)),
                                rhs=wnS(c, hh),
                                start=(c == 0), stop=(c == DC - 1),
                            )
                        if hh == 0:
                            nc.vector.tensor_copy(
                                vt[:, 512 * hh:512 * (hh + 1)], ps[:])
                        else:
                            nc.scalar.copy(
                                vt[:, 512 * hh:512 * (hh + 1)], ps[:])
                    return vt

                # --- out = p @ vout: psum [128 q-tok, 512 d] natural;
                # each q-pack's out is one 256KB fully-contiguous DMA

                def _pv(qp, vt):
                    o = out_pool.tile([128, D], F16, tag="o", name="o")
                    r0 = (b * QP + qp) * 128
                    for hh in range(2):
                        ps = psB.tile([128, 512], F32, tag="psB")
                        nc.tensor.matmul(
                            ps[:],
                            lhsT=pt_sb[qp][:],
                            rhs=vt[:, 512 * hh:512 * (hh + 1)],
                            start=True, stop=True,
                        )
                        sl = slice(512 * hh, 512 * (hh + 1))
                        if hh == 0:
                            nc.scalar.copy(o[:, sl], ps[:])
                        else:
                            nc.vector.tensor_copy(o[:, sl], ps[:])
                    if b == NBLK - 1 and qp >= 2:
                        # tail: four 64-row fully-contiguous descriptors
                        # across both fast rings so the final transfers
                        # drain in parallel DGE channels
                        nc.sync.dma_start(ot[r0:r0 + 64, :], o[0:64, :])
                        nc.scalar.dma_start(ot[r0 + 64:r0 + 128, :],
                                            o[64:128, :])
                    else:
                        dmae = (nc.scalar if b == NBLK - 1 and qp == 0
                                else nc.sync)
                        dmae.dma_start(ot[r0:r0 + 128, :], o[:])

                if b < NBLK - 1:
                    v_sb = [_vout(tch) for tch in range(QP)]
                    for qp in range(QP):
                        _pv(qp, v_sb[qp])
                else:
                    # trail pv behind vout so each pv's vout evict is
                    # long done and the output DMAs stagger
                    v_sb = [_vout(0), _vout(1)]
                    _pv(0, v_sb[0])
                    v_sb.append(_vout(2))
                    v_sb.append(_vout(3))
                    _pv(1, v_sb[1])
                    _pv(2, v_sb[2])
                    _pv(3, v_sb[3])

    nc.compile()
    return nc


def _make_mask():
    """One [128, 128] additive-mask tile shared by every q-pack: rows
    and columns are the pack's own 4 groups x 32 positions; the group-
    diagonal blocks carry the causal mask, everything else NEG
    (-> exp == 0 exactly)."""
    m = np.full((128, 128), NEG, dtype=np.float32)
    for i in range(4):
        for q in range(NB):
            m[32 * i + q, 32 * i:32 * i + q + 1] = 0.0
    return m


def run(x, Wqkv, bqkv, Wout, bout, trace=False):
    x = np.asarray(x, dtype=np.float32)
    Wqkv = np.asarray(Wqkv, dtype=np.float32)
    bqkv = np.asarray(bqkv, dtype=np.float32)
    Wout = np.asarray(Wout, dtype=np.float32)
    bout = np.asarray(bout, dtype=np.float32)

    Wq, Wk, Wv = Wqkv[:, :D], Wqkv[:, D:2 * D], Wqkv[:, 2 * D:]
    # j-strip layout [128, DC*D]: element (p, 1024j + 128c + d)
    #   = M[128c + p, 128j + d] -- strip j holds every input chunk's
    #   weights for output chunk j
    wm = np.ascontiguousarray(
        (Wq @ Wk.T).reshape(DC, 128, DC, 128).transpose(1, 2, 0, 3)
        .reshape(128, DC * D)).astype(np.float16)
    wn = np.ascontiguousarray(
        (Wv @ Wout).reshape(DC, 128, D).transpose(1, 0, 2)
        .reshape(128, DC * D)).astype(np.float16)
    bq_eff = np.ascontiguousarray(Wk @ bqkv[:D])
    bo_eff = (bout + bqkv[2 * D:] @ Wout).astype(np.float32)
    mask = _make_mask()

    # (B, S, D) -> (group, nb, D), group = b*BL + bl, token = g*NB + nb
    xg = x.reshape(B, NB, BL, D).transpose(0, 2, 1, 3).reshape(NGROUP, NB, D)

    nc = _get_program()
    in_maps = []
    for i in range(N_CORES):
        xt_i = xg[GPC * i:GPC * (i + 1)].reshape(TOK, D).T
        # [NBLK*128, DC*BLK]: row (128b + p), col (512c + t)
        #   = x^T[128c + p, 512b + t]
        xt_i = np.ascontiguousarray(
            xt_i.reshape(DC, 128, NBLK, BLK).transpose(2, 1, 0, 3)
            .reshape(NBLK * 128, XW)).astype(np.float16)
        in_maps.append({
            "xt": xt_i, "wm": wm, "wn": wn, "bq": bq_eff, "mask": mask,
        })
    res = run_bass_kernel_spmd(nc, in_maps, list(range(N_CORES)), trace=trace)

    outs = np.empty((NGROUP, NB, D), dtype=np.float32)
    for i in range(N_CORES):
        outs[GPC * i:GPC * (i + 1)] = (
            res.results[i]["ot"].astype(np.float32).reshape(GPC, NB, D))
    out = (outs.reshape(B, BL, NB, D).transpose(0, 2, 1, 3)
           .reshape(B, S, D)) + bo_eff
    return out, res


def kernel(x, Wqkv, bqkv, Wout, bout):
    out, _ = run(x, Wqkv, bqkv, Wout, bout, trace=False)
    return out


# revision 26
# speedup vs baseline: 1.0157x; 1.0157x over previous
"""Factored (column) attention kernel for Trainium2, 8 NeuronCores.

Reference computation (B=4, S=4096, D=1024, BLOCK_LEN=128, NB=32):
    qkv = x @ Wqkv + bqkv ; split q,k,v
    'column' attention: each (batch, within-block position bl) row attends
    causally over the NB=32 block indices -> 512 independent length-32
    single-head attentions with head dim 1024.
    out = attn @ Wout + bout

Algebraic fold (halves device matmul work vs the 4-GEMM formulation):
  scores = (x Wq + bq) . (x Wk)  =  (x M + bq_eff) . x,  M = Wq Wk^T,
           bq_eff = Wk bq           (bk cancels in softmax)
  out    = p @ (x Wv + bv) Wout + bout = p @ (x N) + bo_eff,
           N = Wv Wout, bo_eff = bout + bv Wout   (rows of p sum to 1)
so the device only computes TWO [tok,1024]x[1024,1024] GEMMs (q'' = x M,
vout = x N) plus the tiny 32-long attentions; k/v/out projections vanish.
M, N, bq_eff, bo_eff are precomputed host-side in fp32.

Sharding: data-parallel over the 512 independent (b, bl) attention rows,
64 rows (2048 tokens) per core.  All operands are staged host-side into
[128, wide] layouts so each DRAM transfer is one large fully-contiguous
DMA (dma_start issue costs ~0.7us of engine time each, and the startup
is HBM-bandwidth-bound, so few+fat+critical-first transfers matter):
  - x^T block-major: one [128, 4096] (=1MB) DMA per 512-token block
  - M, N: [128, 8192] tiles, DMA'd in quarters/halves interleaved with
    x^T block 0 across the three DMA-capable queues (sync/act/gpsimd),
    critical first; subtile deps let the PE start on the first quarter
  - q'' produced in transposed layout [D, tok] (lhsT = M chunk)
  - scores for a 4-group q-pack: one [K=128,M=128,N=128] matmul per
    d-chunk of q''^T against the SAME x^T chunk (no k projection)
  - softmax on [128,128] tiles; exp+rowsum fused via accum_out;
    normalized p transposed by one DVE stream-transpose
  - vout = x N in natural layout [tok, D] (lhsT = x^T chunk)
  - out = p @ vout via lhsT = p^T: psum [128 q-tok, 512 d] natural, so
    each q-pack's output is one 256KB fully-contiguous DMA; bo_eff is
    added host-side
Numerics: all matmul operands fp16 (fp32 PSUM accumulation); host-
simulated end-to-end rms error vs the fp32 reference is ~5.3e-4.
"""

import numpy as np

import concourse.bacc as bacc
import concourse.mybir as mybir
import concourse.tile as tile
from concourse.bass_utils import run_bass_kernel_spmd

N_CORES = 8
B, S, D = 4, 4096, 1024
BL = 128          # BLOCK_LEN (within-block positions)
NB = S // BL      # 32 block indices = attention sequence length
NGROUP = B * BL   # 512 independent attention rows
GPC = NGROUP // N_CORES   # 64 groups per core
TOK = GPC * NB    # 2048 tokens per core
BLK = 512         # tokens per fused block (16 groups, 4 q-packs)
NBLK = TOK // BLK  # 4
QP = BLK // 128   # q-packs per block
DC = D // 128     # 8 d-chunks
XW = DC * BLK     # 4096 cols of one x^T block tile
SCALE = 1.0 / np.sqrt(D)
NEG = -1.0e30
WARMUP = 26       # PE warm-up matmuls covering the first weight DMAs

F32 = mybir.dt.float32
F16 = mybir.dt.float16

_PROGRAM = None


def _get_program():
    global _PROGRAM
    if _PROGRAM is None:
        _PROGRAM = _build_program()
    return _PROGRAM


def _build_program():
    nc = bacc.Bacc("TRN2", target_bir_lowering=False, debug=False,
                   num_devices=N_CORES)
    xt = nc.dram_tensor("xt", [NBLK * 128, XW], F16,
                        kind="ExternalInput").ap()
    wm = nc.dram_tensor("wm", [128, DC * D], F16, kind="ExternalInput").ap()
    wn = nc.dram_tensor("wn", [128, DC * D], F16, kind="ExternalInput").ap()
    bq = nc.dram_tensor("bq", [D], F32, kind="ExternalInput").ap()
    mask = nc.dram_tensor("mask", [128, 128], F32,
                          kind="ExternalInput").ap()
    ot = nc.dram_tensor("ot", [TOK, D], F16, kind="ExternalOutput").ap()

    with tile.TileContext(nc) as tc:
        with (
            tc.tile_pool(name="w", bufs=1) as w_pool,
            tc.tile_pool(name="const", bufs=1) as const,
            tc.tile_pool(name="xt", bufs=4) as xt_pool,
            tc.tile_pool(name="q", bufs=2) as q_pool,
            tc.tile_pool(name="v", bufs=5) as v_pool,
            tc.tile_pool(name="sm", bufs=6) as sm_pool,
            tc.tile_pool(name="smh", bufs=8) as smh_pool,
            tc.tile_pool(name="small", bufs=8) as small_pool,
            tc.tile_pool(name="out", bufs=3) as out_pool,
            tc.tile_pool(name="psA", bufs=5, space="PSUM") as psA,
            tc.tile_pool(name="psB", bufs=3, space="PSUM") as psB,
        ):
            # --- staged input DMAs.  warm-up matmuls on a zeroed tile
            # keep the PE busy (and its clock ramped up) while the first
            # weight DMAs land
            wu = const.tile([128, 512], F16, tag="warm")
            nc.vector.memset(wu[:], 0.0)
            wu_ps = psB.tile([128, 512], F32, tag="psB", name="wu_ps")
            for _ in range(WARMUP):
                nc.tensor.matmul(wu_ps[:], lhsT=wu[:, 0:128], rhs=wu[:],
                                 start=True, stop=True)
            # critical bytes (M + x^T block 0 = 3MB) first, ~1MB per DMA
            # queue.  M is staged host-side in j-strip order so each
            # 256KB strip completes one q'' output chunk's weights; the
            # PE's j-psum pipeline starts after strip 0 + x^T block 0
            # instead of after the whole 2MB of M.
            wm_sb = w_pool.tile([128, DC * D], F16, tag="wm", name="wm_sb")
            wn_sb = w_pool.tile([128, DC * D], F16, tag="wn", name="wn_sb")
            xt_sbs = [xt_pool.tile([128, XW], F16, tag="xt", name=f"xt{b}")
                      for b in range(NBLK)]
            mask_sb = const.tile([128, 128], F32, tag="mask")
            bq_sb = const.tile([128, DC], F32, tag="bq")
            # gpsimd's SWDGE ring is ~4x slower than the sync/scalar
            # HWDGE rings -- it only gets the tiny constants; everything
            # else rides the two fast rings, critical bytes first
            nc.gpsimd.dma_start(mask_sb[:], mask[:])
            nc.gpsimd.dma_start(bq_sb[:], bq.rearrange("(c p) -> p c", p=128))
            nc.sync.dma_start(xt_sbs[0][:, 0:XW // 2],
                              xt[0:128, 0:XW // 2])
            nc.scalar.dma_start(xt_sbs[0][:, XW // 2:XW],
                                xt[0:128, XW // 2:XW])
            # strip 0 split across both rings: the first q'' psum only
            # waits for x^T block 0 + 128KB per ring beyond it
            nc.sync.dma_start(wm_sb[:, 0:D // 2], wm[:, 0:D // 2])
            nc.scalar.dma_start(wm_sb[:, D // 2:D], wm[:, D // 2:D])
            for j in range(1, DC):
                eng = nc.sync if j % 2 == 0 else nc.scalar
                eng.dma_start(wm_sb[:, D * j:D * (j + 1)],
                              wm[:, D * j:D * (j + 1)])
            # non-critical: N halves, x^T block 1 (blocks 2,3 prefetch
            # from inside the block loop)
            H = DC * D // 2
            nc.sync.dma_start(wn_sb[:, 0:H], wn[:, 0:H])
            nc.scalar.dma_start(wn_sb[:, H:2 * H], wn[:, H:2 * H])
            nc.sync.dma_start(xt_sbs[1][:, 0:XW // 2], xt[128:256, 0:XW // 2])
            nc.scalar.dma_start(xt_sbs[1][:, XW // 2:XW],
                                xt[128:256, XW // 2:XW])

            def wmS(c, j):
                return wm_sb[:, 1024 * j + 128 * c:1024 * j + 128 * (c + 1)]

            def wnS(c, h):
                return wn_sb[:, 1024 * c + 512 * h:1024 * c + 512 * (h + 1)]

            for b in range(NBLK):
                if b + 2 < NBLK:
                    r0 = 128 * (b + 2)
                    nc.sync.dma_start(xt_sbs[b + 2][:, 0:XW // 2],
                                      xt[r0:r0 + 128, 0:XW // 2])
                    nc.scalar.dma_start(xt_sbs[b + 2][:, XW // 2:XW],
                                        xt[r0:r0 + 128, XW // 2:XW])
                xt_sb = xt_sbs[b]

                def xtS(c, lo, hi):
                    return xt_sb[:, 512 * c + lo:512 * c + hi]

                # --- q''^T projection: psum [dout-chunk 128, BLK tok]
                q_sb = []
                for j in range(DC):
                    ps = psA.tile([128, BLK], F32, tag="psA")
                    for c in range(DC):
                        nc.tensor.matmul(
                            ps[:], lhsT=wmS(c, j), rhs=xtS(c, 0, BLK),
                            start=(c == 0), stop=(c == DC - 1),
                        )
                    q = q_pool.tile([128, BLK], F16, tag=f"q{j}",
                                    name=f"q{j}")
                    nc.scalar.add(q[:], ps[:], bq_sb[:, j:j + 1])
                    q_sb.append(q)

                # --- scores + softmax per 4-group q-pack (before the
                # vout projection so the softmax chain hides behind it)
                pt_sb = []
                for qp in range(QP):
                    ps = psB.tile([128, 128], F32, tag="psB")
                    for c in range(DC):
                        nc.tensor.matmul(
                            ps[:],
                            lhsT=q_sb[c][:, 128 * qp:128 * (qp + 1)],
                            rhs=xtS(c, 128 * qp, 128 * (qp + 1)),
                            start=(c == 0), stop=(c == DC - 1),
                        )
                    tm = sm_pool.tile([128, 128], F32, tag="sm")
                    nc.vector.tensor_add(tm[:], ps[:], mask_sb[:])
                    p4 = sm_pool.tile([128, 128], F32, tag="sm")
                    s4 = small_pool.tile([128, 1], F32, tag="s4")
                    nc.scalar.activation(
                        p4[:], tm[:], mybir.ActivationFunctionType.Exp,
                        scale=float(SCALE), accum_out=s4[:],
                    )
                    r4 = small_pool.tile([128, 1], F32, tag="r4")
                    nc.vector.reciprocal(r4[:], s4[:])
                    pn = smh_pool.tile([128, 128], F16, tag="smh")
                    nc.vector.tensor_scalar_mul(pn[:], p4[:], r4[:])
                    pt = smh_pool.tile([128, 128], F16, tag="smh")
                    nc.vector.transpose(pt[:], pn[:])
                    pt_sb.append(pt)

                # --- vout = x N natural: psum [tok-chunk 128, 512 dout]
                def _vout(tch):
                    vt = v_pool.tile([128, D], F16, tag="v", name="vt")
                    for hh in range(2):
                        ps = psA.tile([128, 512], F32, tag="psA")
                        for c in range(DC):
                            nc.tensor.matmul(
                                ps[:],
                                lhsT=xtS(c, 128 * tch, 128 * (tch + 1)),
                                rhs=wnS(c, hh),
                                start=(c == 0), stop=(c == DC - 1),
                            )
                        if hh == 0:
                            nc.vector.tensor_copy(
                                vt[:, 512 * hh:512 * (hh + 1)], ps[:])
                        else:
                            nc.scalar.copy(
                                vt[:, 512 * hh:512 * (hh + 1)], ps[:])
                    return vt

                # --- out = p @ vout: psum [128 q-tok, 512 d] natural;
                # each q-pack's out is one 256KB fully-contiguous DMA

                def _pv(qp, vt):
                    o = out_pool.tile([128, D], F16, tag="o", name="o")
                    r0 = (b * QP + qp) * 128
                    for hh in range(2):
                        ps = psB.tile([128, 512], F32, tag="psB")
                        nc.tensor.matmul(
                            ps[:],
                            lhsT=pt_sb[qp][:],
                            rhs=vt[:, 512 * hh:512 * (hh + 1)],
                            start=True, stop=True,
                        )
                        sl = slice(512 * hh, 512 * (hh + 1))
                        if hh == 0:
                            nc.scalar.copy(o[:, sl], ps[:])
                        else:
                            nc.vector.tensor_copy(o[:, sl], ps[:])
                    if b == NBLK - 1 and qp >= 2:
                        # tail: half-tile DMAs on both fast rings so the
                        # final transfer after the last evict is only
                        # 128KB per ring
                        nc.sync.dma_start(ot[r0:r0 + 128, 0:512],
                                          o[:, 0:512])
                        nc.scalar.dma_start(ot[r0:r0 + 128, 512:1024],
                                            o[:, 512:1024])
                    else:
                        dmae = (nc.scalar if b == NBLK - 1 and qp == 0
                                else nc.sync)
                        dmae.dma_start(ot[r0:r0 + 128, :], o[:])

                if b < NBLK - 1:
                    v_sb = [_vout(tch) for tch in range(QP)]
                    for qp in range(QP):
                        _pv(qp, v_sb[qp])
                else:
                    # trail pv behind vout so each pv's vout evict is
                    # long done and the output DMAs stagger
                    v_sb = [_vout(0), _vout(1)]
                    _pv(0, v_sb[0])
                    v_sb.append(_vout(2))
                    v_sb.append(_vout(3))
                    _pv(1, v_sb[1])
                    _pv(2, v_sb[2])
                    _pv(3, v_sb[3])

    nc.compile()
    return nc


def _make_mask():
    """One [128, 128] additive-mask tile shared by every q-pack: rows
    and columns are the pack's own 4 groups x 32 positions; the group-
    diagonal blocks carry the causal mask, everything else NEG
    (-> exp == 0 exactly)."""
    m = np.full((128, 128), NEG, dtype=np.float32)
    for i in range(4):
        for q in range(NB):
            m[32 * i + q, 32 * i:32 * i + q + 1] = 0.0
    return m


def run(x, Wqkv, bqkv, Wout, bout, trace=False):
    x = np.asarray(x, dtype=np.float32)
    Wqkv = np.asarray(Wqkv, dtype=np.float32)
    bqkv = np.asarray(bqkv, dtype=np.float32)
    Wout = np.asarray(Wout, dtype=np.float32)
    bout = np.asarray(bout, dtype=np.float32)

    Wq, Wk, Wv = Wqkv[:, :D], Wqkv[:, D:2 * D], Wqkv[:, 2 * D:]
    # j-strip layout [128, DC*D]: element (p, 1024j + 128c + d)
    #   = M[128c + p, 128j + d] -- strip j holds every input chunk's
    #   weights for output chunk j
    wm = np.ascontiguousarray(
        (Wq @ Wk.T).reshape(DC, 128, DC, 128).transpose(1, 2, 0, 3)
        .reshape(128, DC * D)).astype(np.float16)
    wn = np.ascontiguousarray(
        (Wv @ Wout).reshape(DC, 128, D).transpose(1, 0, 2)
        .reshape(128, DC * D)).astype(np.float16)
    bq_eff = np.ascontiguousarray(Wk @ bqkv[:D])
    bo_eff = (bout + bqkv[2 * D:] @ Wout).astype(np.float32)
    mask = _make_mask()

    # (B, S, D) -> (group, nb, D), group = b*BL + bl, token = g*NB + nb
    xg = x.reshape(B, NB, BL, D).transpose(0, 2, 1, 3).reshape(NGROUP, NB, D)

    nc = _get_program()
    in_maps = []
    for i in range(N_CORES):
        xt_i = xg[GPC * i:GPC * (i + 1)].reshape(TOK, D).T
        # [NBLK*128, DC*BLK]: row (128b + p), col (512c + t)
        #   = x^T[128c + p, 512b + t]
        xt_i = np.ascontiguousarray(
            xt_i.reshape(DC, 128, NBLK, BLK).transpose(2, 1, 0, 3)
            .reshape(NBLK * 128, XW)).astype(np.float16)
        in_maps.append({
            "xt": xt_i, "wm": wm, "wn": wn, "bq": bq_eff, "mask": mask,
        })
    res = run_bass_kernel_spmd(nc, in_maps, list(range(N_CORES)), trace=trace)

    outs = np.empty((NGROUP, NB, D), dtype=np.float32)
    for i in range(N_CORES):
        outs[GPC * i:GPC * (i + 1)] = (
            res.results[i]["ot"].astype(np.float32).reshape(GPC, NB, D))
    out = (outs.reshape(B, BL, NB, D).transpose(0, 2, 1, 3)
           .reshape(B, S, D)) + bo_eff
    return out, res


def kernel(x, Wqkv, bqkv, Wout, bout):
    out, _ = run(x, Wqkv, bqkv, Wout, bout, trace=False)
    return out
